# revision 1
# baseline (speedup 1.0000x reference)
"""Trainium2 Bass kernel for CustomizeL2Loss.

Reference computation (x, y: (N, C, T, V, M) = (256, 3, 600, 25, 2) f32):
    motion    = x[:, :, 1:] - x[:, :, :-1]
    mean_move = mean(|motion|, axis=(C, T-1, M))            -> (N, V)
    ratio     = V * mean_move / sum_v(mean_move)            -> (N, V)
    loss      = mean((x - y)**2 * ratio[:, None, None, :, None])

Decomposition used here (linearity):
    S[n, v] = sum_{c,t,m} (x - y)^2          A[n, v] = sum_{c,t,m} |motion|
    loss = (1 / (N*C*T*V*M)) * sum_n V * sum_v(A[n,v] * S[n,v]) / sum_v A[n,v]
(the 1/(C*(T-1)*M) mean_move normalization cancels inside ratio)

Device kernel (data-parallel over batch, 8 cores x 32 samples):
  Per-core layout: x viewed as (96 blocks, 120, 250) where block = (n_local, c),
  partition row p = t-group [5p, 5p+5), free = (t_sub 5, v*m 50). The host
  pretransposes to (P, NBLK, 2, FB) so each tile load is one HWDGE-sized DMA
  of fully contiguous per-partition runs; the load itself is a SWDGE cast-DMA
  (fp32 HBM -> bf16 SBUF) so all DVE tensor_tensor ops run in 2x packed mode.
  Per tile: d = x - y (DVE), sq = Square(d) (ScalarE, bf16), in-row motion
  pairs d1 (DVE), cross-row pairs via GpSimd strip copy + SBUF->SBUF
  partition-shift DMA + GpSimd subtract, |.| on ScalarE. Reduction over (c, t)
  via TensorE: replicated-ones (K, 32) stationary, data as moving rhs, PSUM
  accumulated per sample at partition strip 32*(s%4), bank s//4, with per-bank
  incremental ScalarE drains; one strided output DMA. The last tile is tapered
  (6+3+3 blocks) to shorten the trailing dependency chain. Host folds
  (t_sub, m) and does the tiny (256, 25) ratio/loss math in float64.
"""

from contextlib import ExitStack

import numpy as np

import concourse.bass as bass
import concourse.bacc as bacc
import concourse.tile as tile
from concourse import mybir
from concourse.bass_utils import run_bass_kernel_spmd

# Problem shape (hardcoded; kernel.py must be self-contained)
N, C, T, V, M = 256, 3, 600, 25, 2
N_CORES = 8
NL = N // N_CORES       # 32 samples per core
NBLK = NL * C           # 96 (n_local, c) blocks per core
P = 120                 # partition rows per block (t-groups)
TS = T // P             # 5 t rows per partition row
FB = TS * V * M         # 250 block free width
W1 = 4 * V * M          # 200 = in-row motion pair columns
WVM = V * M             # 50

B_BLK = 12              # blocks per tile (= 4 samples)
N_TILES = NBLK // B_BLK  # 8
SPT = B_BLK // C        # 4 samples per tile -> 4 partition strips
BANK = 512              # fp32 per PSUM bank per partition
OUTW = FB + W1 + WVM    # 500 used columns per sample

F32 = mybir.dt.float32
BF16 = mybir.dt.bfloat16


def build_program(reps: int = 1, io_bufs: int = 4, work_bufs: int = 2,
                  taper=(6, 3, 3), gp_shift: bool = True):
    """reps > 1 repeats the full pass in-NEFF (benchmarking only: amortizes
    host dispatch overhead; outputs are identical every rep).

    taper splits the last 12-block tile so the trailing dependency chain
    (load -> subs -> shift-DMA -> abs -> matmul -> drain) is short.
    gp_shift puts the strip copy + d2 subtract on GpSimd, relieving DVE.
    """
    segs = []
    for i in range(N_TILES):
        segs.append((i, i * B_BLK, B_BLK))
    if taper:
        last = segs.pop()
        off = 0
        for t in taper:
            segs.append((last[0], last[1] + off, t))
            off += t
        assert off == B_BLK

    nc = bacc.Bacc("TRN2", target_bir_lowering=False, debug=False)
    # Host pretransposes so the DRAM layout is (P, NBLK, 2, FB): every tile
    # load is ONE dma_start (single completion semaphore) of 120 partitions x
    # one fully contiguous nb*2*250-element run each.
    xy_d = nc.dram_tensor("xy", [P, NBLK, 2, FB], F32, kind="ExternalInput").ap()
    out_d = nc.dram_tensor("out", [SPT, N_TILES, OUTW], F32, kind="ExternalOutput").ap()

    with tile.TileContext(nc) as tc, ExitStack() as ctx:
        io = ctx.enter_context(tc.tile_pool(name="io", bufs=io_bufs))
        shift = ctx.enter_context(tc.tile_pool(name="shift", bufs=work_bufs))
        work = ctx.enter_context(tc.tile_pool(name="work", bufs=work_bufs))
        singles = ctx.enter_context(tc.tile_pool(name="singles", bufs=1))
        psum_pool = ctx.enter_context(tc.tile_pool(name="psum", bufs=1, space="PSUM"))

        ones_t = singles.tile([P, 32], BF16)
        nc.vector.memset(ones_t, 1.0)
        psum_t = psum_pool.tile([128, N_TILES, BANK], F32)
        collect = singles.tile([128, N_TILES, OUTW], F32)

        eng = nc.gpsimd if gp_shift else nc.vector
        for _ in range(reps):
            for (bank, b0, nb) in segs:
                ns = nb // C
                j0 = (b0 - bank * B_BLK) // C
                # SWDGE cast-DMA: HBM fp32 -> SBUF bf16. HBM read bytes are
                # unchanged, but every DVE tensor_tensor downstream gets the
                # 2x_1P packed mode and SBUF footprint halves.
                xy_t = io.tile([P, nb, 2, FB], BF16, tag="xy")
                nc.gpsimd.dma_start(out=xy_t, in_=xy_d[:, b0 : b0 + nb])
                x_t = xy_t[:, :, 0]
                y_t = xy_t[:, :, 1]

                d_t = work.tile([P, nb, FB], BF16, tag="d")
                nc.vector.tensor_sub(d_t, x_t, y_t)
                sq_t = work.tile([P, nb, FB], BF16, tag="sq")
                nc.scalar.activation(sq_t, d_t, mybir.ActivationFunctionType.Square)

                d1_t = work.tile([P, nb, W1], BF16, tag="d1")
                nc.vector.tensor_sub(d1_t, x_t[:, :, WVM:FB], x_t[:, :, 0:W1])
                a1_t = work.tile([P, nb, W1], BF16, tag="a1")
                nc.scalar.activation(a1_t, d1_t, mybir.ActivationFunctionType.Abs)

                # strip copy keeps the xy tile's readers on compute engines;
                # the t-row shift rides a small SBUF->SBUF DMA off the strip.
                strip_t = work.tile([P, nb, WVM], BF16, tag="strip")
                eng.tensor_copy(strip_t, x_t[:, :, 0:WVM])
                xs_t = shift.tile([P - 1, nb, WVM], BF16, tag="xs")
                nc.sync.dma_start(out=xs_t, in_=strip_t[1:P])
                d2_t = work.tile([P - 1, nb, WVM], BF16, tag="d2")
                eng.tensor_sub(d2_t, xs_t, x_t[0 : P - 1, :, FB - WVM : FB])
                a2_t = work.tile([P - 1, nb, WVM], BF16, tag="a2")
                nc.scalar.activation(a2_t, d2_t, mybir.ActivationFunctionType.Abs)

                for j in range(ns):
                    sp = 32 * (j0 + j)
                    for (rhs, lo, hi, kk) in (
                        (sq_t, 0, FB, P),
                        (a1_t, FB, FB + W1, P),
                        (a2_t, FB + W1, OUTW, P - 1),
                    ):
                        for c in range(C):
                            bb = C * j + c
                            nc.tensor.matmul(
                                psum_t[sp : sp + 32, bank, lo:hi],
                                ones_t[0:kk, 0:32],
                                rhs[:, bb, :],
                                start=(c == 0),
                                stop=(c == C - 1),
                                tile_position=(0, sp),
                            )
                if b0 + nb == (bank + 1) * B_BLK:
                    # bank complete -> drain it (overlaps later iterations)
                    nc.scalar.copy(collect[:, bank, :], psum_t[:, bank, 0:OUTW])

        nc.sync.dma_start(out=out_d, in_=collect[0:128:32])

    nc.compile()
    return nc


def host_reduce(outs: np.ndarray) -> np.float32:
    """outs: (N_CORES, SPT, N_TILES, OUTW) f32 -> scalar loss."""
    o = outs.astype(np.float64)
    # sample n = 32*core + 4*i + j  <->  o[core, j, i]
    S = o[..., 0:FB].reshape(N_CORES, SPT, N_TILES, TS, V, M).sum(axis=(3, 5))
    A = o[..., FB : FB + W1].reshape(N_CORES, SPT, N_TILES, 4, V, M).sum(axis=(3, 5))
    A += o[..., FB + W1 : OUTW].reshape(N_CORES, SPT, N_TILES, V, M).sum(axis=4)
    num = (A * S).sum(axis=-1)
    den = A.sum(axis=-1)
    loss = (V * num / den).sum() / float(N * C * T * V * M)
    return np.float32(loss)


def make_in_maps(x, y):
    # (N,C,T,V,M) -> per core (NBLK, P, FB); stack x/y per block; move the
    # partition axis outermost so device DMAs read contiguous runs.
    xr = x.reshape(N_CORES, NBLK, P, FB)
    yr = y.reshape(N_CORES, NBLK, P, FB)
    xy = np.stack([xr, yr], axis=2)            # (cores, NBLK, 2, P, FB)
    xy = np.ascontiguousarray(xy.transpose(0, 3, 1, 2, 4))  # (cores, P, NBLK, 2, FB)
    return [{"xy": xy[k]} for k in range(N_CORES)]


_NC_CACHE = None


def kernel(x: np.ndarray, y: np.ndarray) -> np.ndarray:
    global _NC_CACHE
    x = np.asarray(x, dtype=np.float32)
    y = np.asarray(y, dtype=np.float32)
    in_maps = make_in_maps(x, y)
    if _NC_CACHE is None:
        _NC_CACHE = build_program()
    res = run_bass_kernel_spmd(_NC_CACHE, in_maps, list(range(N_CORES))).results
    outs = np.stack([res[k]["out"] for k in range(N_CORES)])
    return host_reduce(outs)

